# revision 7
# baseline (speedup 1.0000x reference)
"""Trainium2 Bass kernel for nn_NeocorticalModule (vq_codebook).

Data-parallel over N across 8 NeuronCores. Host slices + pre-transposes the
episodic traces (layout choice for the shard), each core computes local
segment sums/counts of the encoded traces, host reduces the tiny [64,33]
partials and applies the schema update.

Per 512-row subtile on device:
  hT[64,512]   = sum_k W1_k.T @ XT_k          (PSUM accumulate over 8 chunks)
  h1[65,512]   = [relu(hT + b1); 1]           (ACT bias+relu, ones row)
  per 128-row group g (stationary = h1 block, loaded once for two matmuls):
    enc[128,32]  = h1_g.T @ [W2; b2]
    sims[128,64] = h1_g.T @ ([W2; b2] @ normalize(schemas).T)
    oh[128,64]   = (sims == rowmax(sims))     (DVE; argmax one-hot)
    acc[64,33]  += oh.T @ [enc | 1]           (PSUM accumulate, whole kernel)
"""

import numpy as np

DIM = 1024
SCHEMA_DIM = 32
NUM_SCHEMAS = 64
LR = 0.01
EPS = 1e-8
N = 131072
NCORES = 8
RPC = N // NCORES          # rows per core = 16384
BLK = 2048                 # rows per DMA block (1 MiB per chunk DMA)
SUB = 512                  # rows per PSUM subtile (max fp32 moving free dim)
G = 128                    # rows per stationary group
KCH = DIM // 128           # 8 contraction chunks
HD = 2 * SCHEMA_DIM        # 64 hidden dim
SC = NUM_SCHEMAS           # 64

_CACHE = {}


def _split_multi_waits(nc):
    """This walrus build's CoreV3 codegen accepts only ONE sync wait per
    instruction. Tile's auto-generated sems can attach several. Hoist the
    extra waits onto single-wait NOPs inserted just before the instruction on
    the same engine (semantically identical: the engine stalls at the same
    point in its stream)."""
    import concourse.mybir as mybir

    counter = [0]

    def process_block(bb):
        insts = bb.instructions
        new_insts = []
        changed = False
        for inst in insts:
            sub_bb = getattr(inst, "body_bb", None)
            if sub_bb is not None:
                process_block(sub_bb)
            si = getattr(inst, "sync_info", None)
            waits = list(si.on_wait) if si is not None and si.on_wait else []
            if len(waits) > 1:
                changed = True
                for w in waits[:-1]:
                    counter[0] += 1
                    nop = mybir.InstNoOp(
                        name=f"waitsplit-{counter[0]}",
                        engine=inst.engine,
                        sync_info=mybir.SyncInfo(on_wait=[w], on_update=[]),
                    )
                    new_insts.append(nop)
                si.on_wait = waits[-1:]
            new_insts.append(inst)
        if changed:
            bb.instructions = new_insts

    for fn in nc.m.functions:
        for bb in fn.blocks:
            process_block(bb)


def _build_nc(rpc=RPC, blk=BLK, split_waits=True):
    import concourse.bass as bass
    import concourse.mybir as mybir
    import concourse.tile as tile
    from concourse.bass import ts

    f32 = mybir.dt.float32
    nblk = rpc // blk
    nsub = blk // SUB
    ng = SUB // G

    nc = bass.Bass(num_devices=NCORES)
    xt_d = nc.dram_tensor("xt", [DIM, rpc], f32, kind="ExternalInput")
    w1s_d = nc.dram_tensor("w1s", [128, KCH * HD], f32, kind="ExternalInput")
    b1c_d = nc.dram_tensor("b1c", [HD, 1], f32, kind="ExternalInput")
    w2a_d = nc.dram_tensor("w2a", [HD + 1, SCHEMA_DIM], f32, kind="ExternalInput")
    wsim_d = nc.dram_tensor("wsim", [HD + 1, SC], f32, kind="ExternalInput")
    part_d = nc.dram_tensor("partial", [SC, SCHEMA_DIM + 1], f32, kind="ExternalOutput")

    with tile.TileContext(nc) as tc:
        with (
            tc.tile_pool(name="wpool", bufs=1) as wpool,
            tc.tile_pool(name="xpool", bufs=2) as xpool,
            tc.tile_pool(name="spool", bufs=4) as spool,
            tc.tile_pool(name="hpsum", bufs=4, space="PSUM") as hpsum,
            tc.tile_pool(name="sepsum", bufs=3, space="PSUM") as sepsum,
            tc.tile_pool(name="accpsum", bufs=1, space="PSUM") as accpsum,
        ):
            w1s = wpool.tile([128, KCH * HD], f32)
            nc.sync.dma_start(w1s[:], w1s_d[:, :])
            b1c = wpool.tile([HD, 1], f32)
            nc.sync.dma_start(b1c[:], b1c_d[:, :])
            w2a = wpool.tile([HD + 1, SCHEMA_DIM], f32)
            nc.sync.dma_start(w2a[:], w2a_d[:, :])
            wsim = wpool.tile([HD + 1, SC], f32)
            nc.sync.dma_start(wsim[:], wsim_d[:, :])

            acc = accpsum.tile([SC, SCHEMA_DIM + 1], f32)
            first = True

            for b in range(nblk):
                xts = []
                for k in range(KCH):
                    xk = xpool.tile([128, blk], f32, tag=f"xt{k}", name=f"xt{k}")
                    nc.sync.dma_start(xk[:], xt_d[ts(k, 128), ts(b, blk)])
                    xts.append(xk)
                for s in range(nsub):
                    ht = hpsum.tile([HD, SUB], f32, tag="ht", name="ht")
                    for k in range(KCH):
                        nc.tensor.matmul(
                            ht[:],
                            lhsT=w1s[:, ts(k, HD)],
                            rhs=xts[k][:, ts(s, SUB)],
                            start=(k == 0),
                            stop=(k == KCH - 1),
                        )
                    h1 = spool.tile([HD + 1, SUB], f32, tag="h1", name="h1")
                    nc.scalar.activation(
                        h1[0:HD, :],
                        ht[:],
                        mybir.ActivationFunctionType.Relu,
                        bias=b1c[:, 0:1],
                        scale=1.0,
                    )
                    nc.gpsimd.memset(h1[HD : HD + 1, :], 1.0)
                    for g in range(ng):
                        last = b == nblk - 1 and s == nsub - 1 and g == ng - 1
                        se = sepsum.tile([128, 128], f32, tag="se", name="se")
                        nc.tensor.matmul(
                            se[:, 0:SCHEMA_DIM],
                            lhsT=h1[:, ts(g, G)],
                            rhs=w2a[:],
                            start=True,
                            stop=True,
                        )
                        nc.tensor.matmul(
                            se[:, 64 : 64 + SC],
                            lhsT=h1[:, ts(g, G)],
                            rhs=wsim[:],
                            start=True,
                            stop=True,
                        )
                        rm = spool.tile([128, 1], f32, tag="rm", name="rm")
                        nc.vector.reduce_max(
                            rm[:, 0:1], se[:, 64 : 64 + SC], axis=mybir.AxisListType.X
                        )
                        oh = spool.tile([128, SC], f32, tag="oh", name="oh")
                        nc.vector.tensor_scalar(
                            oh[:],
                            se[:, 64 : 64 + SC],
                            rm[:, 0:1],
                            None,
                            op0=mybir.AluOpType.is_equal,
                        )
                        enc1 = spool.tile(
                            [128, SCHEMA_DIM + 1], f32, tag="enc1", name="enc1"
                        )
                        nc.scalar.copy(enc1[:, 0:SCHEMA_DIM], se[:, 0:SCHEMA_DIM])
                        nc.gpsimd.memset(enc1[:, SCHEMA_DIM : SCHEMA_DIM + 1], 1.0)
                        nc.tensor.matmul(
                            acc[:, 0 : SCHEMA_DIM + 1],
                            lhsT=oh[:],
                            rhs=enc1[:],
                            start=first,
                            stop=last,
                        )
                        first = False

            out_sb = wpool.tile([SC, SCHEMA_DIM + 1], f32)
            nc.scalar.copy(out_sb[:], acc[:])
            nc.sync.dma_start(part_d[:, :], out_sb[:])

    if split_waits:
        _split_multi_waits(nc)
    return nc


def _get_nc():
    if "nc" not in _CACHE:
        _CACHE["nc"] = _build_nc()
    return _CACHE["nc"]


def _transpose_shard(x):
    """[R, DIM] row-major -> [DIM, R] contiguous, cache-blocked."""
    r = x.shape[0]
    out = np.empty((DIM, r), dtype=np.float32)
    step = 512
    for i in range(0, r, step):
        out[:, i : i + step] = x[i : i + step, :].T
    return out


def _prep_weights(W1, b1, W2, b2, schemas):
    w1s = np.ascontiguousarray(
        W1.reshape(KCH, 128, HD).transpose(1, 0, 2).reshape(128, KCH * HD)
    )
    b1c = np.ascontiguousarray(b1.reshape(HD, 1))
    w2a = np.ascontiguousarray(np.concatenate([W2, b2[None, :]], axis=0))
    s64 = schemas.astype(np.float64)
    norms = np.maximum(np.sqrt((s64 * s64).sum(axis=1, keepdims=True)), EPS)
    schn = s64 / norms
    wsim = (w2a.astype(np.float64) @ schn.T).astype(np.float32)
    return w1s, b1c, w2a, wsim


def run_device(episodic_traces, W1, b1, W2, b2, schemas, **run_kwargs):
    """Compile + run the SPMD kernel; returns (partials [8,64,33], results)."""
    from concourse.bass_utils import run_bass_kernel_spmd

    nc = _get_nc()
    w1s, b1c, w2a, wsim = _prep_weights(W1, b1, W2, b2, schemas)
    in_maps = []
    for c in range(NCORES):
        xt = _transpose_shard(episodic_traces[c * RPC : (c + 1) * RPC])
        in_maps.append({"xt": xt, "w1s": w1s, "b1c": b1c, "w2a": w2a, "wsim": wsim})
    res = run_bass_kernel_spmd(nc, in_maps, list(range(NCORES)), **run_kwargs)
    partials = np.stack(
        [np.asarray(res.results[i]["partial"]) for i in range(NCORES)]
    )
    return partials, res


def kernel(episodic_traces, W1, b1, W2, b2, schemas, schema_usage):
    episodic_traces = np.asarray(episodic_traces, np.float32)
    W1 = np.asarray(W1, np.float32)
    b1 = np.asarray(b1, np.float32)
    W2 = np.asarray(W2, np.float32)
    b2 = np.asarray(b2, np.float32)
    schemas = np.asarray(schemas, np.float32)
    schema_usage = np.asarray(schema_usage, np.float32)

    partials, _ = run_device(episodic_traces, W1, b1, W2, b2, schemas)

    sums = partials[:, :, :SCHEMA_DIM].sum(axis=0)
    counts = partials[:, :, SCHEMA_DIM].sum(axis=0)

    nonempty = counts > 0
    target = sums / np.maximum(counts, np.float32(1.0))[:, None]
    delta = np.float32(LR) * (target - schemas) * nonempty[:, None].astype(np.float32)
    new_schemas = schemas + delta
    new_usage = schema_usage + counts
    norms = np.sqrt((delta * delta).sum(axis=-1))
    num_updated = np.int32(nonempty.sum())
    mean_update_norm = np.float32(norms.sum() / np.float32(max(1, int(num_updated))))
    return new_schemas, new_usage, num_updated, mean_update_norm


# revision 11
# speedup vs baseline: 589.6170x; 589.6170x over previous
"""Trainium2 Bass kernel for nn_NeocorticalModule (vq_codebook).

Data-parallel over N across 8 NeuronCores. Host slices + pre-transposes the
episodic traces (layout choice for the shard), each core computes local
segment sums/counts of the encoded traces, host reduces the tiny [64,33]
partials and applies the schema update.

Per 512-row subtile on device:
  hT[64,512]   = sum_k W1_k.T @ XT_k          (PSUM accumulate over 8 chunks)
  h1[65,512]   = [relu(hT + b1); 1]           (ACT bias+relu, ones row)
  per 128-row group g (stationary = h1 block, loaded once for two matmuls):
    enc[128,32]  = h1_g.T @ [W2; b2]
    sims[128,64] = h1_g.T @ ([W2; b2] @ normalize(schemas).T)
    oh[128,64]   = (sims == rowmax(sims))     (DVE; argmax one-hot)
    acc[64,33]  += oh.T @ [enc | 1]           (PSUM accumulate, whole kernel)
"""

import numpy as np

DIM = 1024
SCHEMA_DIM = 32
NUM_SCHEMAS = 64
LR = 0.01
EPS = 1e-8
N = 131072
NCORES = 8
RPC = N // NCORES          # rows per core = 16384
BLK = 2048                 # rows per DMA block (1 MiB per chunk DMA)
SUB = 512                  # rows per PSUM subtile (max fp32 moving free dim)
G = 128                    # rows per stationary group
KCH = DIM // 128           # 8 contraction chunks
HD = 2 * SCHEMA_DIM        # 64 hidden dim
SC = NUM_SCHEMAS           # 64

_CACHE = {}


def _split_multi_waits(nc):
    """This walrus build's CoreV3 codegen accepts only ONE sync wait per
    instruction. Tile's auto-generated sems can attach several. Hoist the
    extra waits onto single-wait NOPs inserted just before the instruction on
    the same engine (semantically identical: the engine stalls at the same
    point in its stream)."""
    import concourse.mybir as mybir

    counter = [0]

    def process_block(bb):
        insts = bb.instructions
        new_insts = []
        changed = False
        for inst in insts:
            sub_bb = getattr(inst, "body_bb", None)
            if sub_bb is not None:
                process_block(sub_bb)
            si = getattr(inst, "sync_info", None)
            waits = list(si.on_wait) if si is not None and si.on_wait else []
            if len(waits) > 1:
                changed = True
                for w in waits[:-1]:
                    counter[0] += 1
                    nop = mybir.InstNoOp(
                        name=f"waitsplit-{counter[0]}",
                        engine=inst.engine,
                        sync_info=mybir.SyncInfo(on_wait=[w], on_update=[]),
                    )
                    new_insts.append(nop)
                si.on_wait = waits[-1:]
            new_insts.append(inst)
        if changed:
            bb.instructions = new_insts

    for fn in nc.m.functions:
        for bb in fn.blocks:
            process_block(bb)


def _build_nc(rpc=RPC, blk=BLK, split_waits=True, reps=1):
    import concourse.bass as bass
    import concourse.mybir as mybir
    import concourse.tile as tile
    from concourse.bass import ts

    f32 = mybir.dt.float32
    nblk = rpc // blk
    nsub = blk // SUB
    ng = SUB // G

    nc = bass.Bass(num_devices=NCORES)
    xt_d = nc.dram_tensor("xt", [DIM, rpc], f32, kind="ExternalInput")
    w1s_d = nc.dram_tensor("w1s", [128, KCH * HD], f32, kind="ExternalInput")
    b1c_d = nc.dram_tensor("b1c", [HD, 1], f32, kind="ExternalInput")
    w2a_d = nc.dram_tensor("w2a", [HD + 1, SCHEMA_DIM], f32, kind="ExternalInput")
    wsim_d = nc.dram_tensor("wsim", [HD + 1, SC], f32, kind="ExternalInput")
    part_d = nc.dram_tensor("partial", [SC, SCHEMA_DIM + 1], f32, kind="ExternalOutput")

    with tile.TileContext(nc) as tc:
        with (
            tc.tile_pool(name="wpool", bufs=1) as wpool,
            tc.tile_pool(name="xpool", bufs=2) as xpool,
            tc.tile_pool(name="spool", bufs=4) as spool,
            tc.tile_pool(name="hpsum", bufs=4, space="PSUM") as hpsum,
            tc.tile_pool(name="sepsum", bufs=3, space="PSUM") as sepsum,
            tc.tile_pool(name="accpsum", bufs=1, space="PSUM") as accpsum,
        ):
            w1s = wpool.tile([128, KCH * HD], f32)
            nc.sync.dma_start(w1s[:], w1s_d[:, :])
            b1c = wpool.tile([HD, 1], f32)
            nc.sync.dma_start(b1c[:], b1c_d[:, :])
            w2a = wpool.tile([HD + 1, SCHEMA_DIM], f32)
            nc.sync.dma_start(w2a[:], w2a_d[:, :])
            wsim = wpool.tile([HD + 1, SC], f32)
            nc.sync.dma_start(wsim[:], wsim_d[:, :])

            acc = accpsum.tile([SC, SCHEMA_DIM + 1], f32)
            first = True

            for bi in range(nblk * reps):
                b = bi % nblk
                last_blk = bi == nblk * reps - 1
                xts = []
                for k in range(KCH):
                    xk = xpool.tile([128, blk], f32, tag=f"xt{k}", name=f"xt{k}")
                    nc.sync.dma_start(xk[:], xt_d[ts(k, 128), ts(b, blk)])
                    xts.append(xk)
                for s in range(nsub):
                    ht = hpsum.tile([HD, SUB], f32, tag="ht", name="ht")
                    for k in range(KCH):
                        nc.tensor.matmul(
                            ht[:],
                            lhsT=w1s[:, ts(k, HD)],
                            rhs=xts[k][:, ts(s, SUB)],
                            start=(k == 0),
                            stop=(k == KCH - 1),
                        )
                    h1 = spool.tile([HD + 1, SUB], f32, tag="h1", name="h1")
                    nc.scalar.activation(
                        h1[0:HD, :],
                        ht[:],
                        mybir.ActivationFunctionType.Relu,
                        bias=b1c[:, 0:1],
                        scale=1.0,
                    )
                    nc.gpsimd.memset(h1[HD : HD + 1, :], 1.0)
                    for g in range(ng):
                        last = last_blk and s == nsub - 1 and g == ng - 1
                        se = sepsum.tile([128, 128], f32, tag="se", name="se")
                        nc.tensor.matmul(
                            se[:, 0:SCHEMA_DIM],
                            lhsT=h1[:, ts(g, G)],
                            rhs=w2a[:],
                            start=True,
                            stop=True,
                        )
                        nc.tensor.matmul(
                            se[:, 64 : 64 + SC],
                            lhsT=h1[:, ts(g, G)],
                            rhs=wsim[:],
                            start=True,
                            stop=True,
                        )
                        rm = spool.tile([128, 1], f32, tag="rm", name="rm")
                        nc.vector.reduce_max(
                            rm[:, 0:1], se[:, 64 : 64 + SC], axis=mybir.AxisListType.X
                        )
                        oh = spool.tile([128, SC], f32, tag="oh", name="oh")
                        nc.vector.tensor_scalar(
                            oh[:],
                            se[:, 64 : 64 + SC],
                            rm[:, 0:1],
                            None,
                            op0=mybir.AluOpType.is_equal,
                        )
                        enc1 = spool.tile(
                            [128, SCHEMA_DIM + 1], f32, tag="enc1", name="enc1"
                        )
                        nc.scalar.copy(enc1[:, 0:SCHEMA_DIM], se[:, 0:SCHEMA_DIM])
                        nc.gpsimd.memset(enc1[:, SCHEMA_DIM : SCHEMA_DIM + 1], 1.0)
                        nc.tensor.matmul(
                            acc[:, 0 : SCHEMA_DIM + 1],
                            lhsT=oh[:],
                            rhs=enc1[:],
                            start=first,
                            stop=last,
                        )
                        first = False

            out_sb = wpool.tile([SC, SCHEMA_DIM + 1], f32)
            nc.scalar.copy(out_sb[:], acc[:])
            nc.sync.dma_start(part_d[:, :], out_sb[:])

    if split_waits:
        _split_multi_waits(nc)
    return nc


def _get_nc():
    if "nc" not in _CACHE:
        _CACHE["nc"] = _build_nc()
    return _CACHE["nc"]


def _transpose_shard(x):
    """[R, DIM] row-major -> [DIM, R] contiguous, cache-blocked."""
    r = x.shape[0]
    out = np.empty((DIM, r), dtype=np.float32)
    step = 512
    for i in range(0, r, step):
        out[:, i : i + step] = x[i : i + step, :].T
    return out


def _prep_weights(W1, b1, W2, b2, schemas):
    w1s = np.ascontiguousarray(
        W1.reshape(KCH, 128, HD).transpose(1, 0, 2).reshape(128, KCH * HD)
    )
    b1c = np.ascontiguousarray(b1.reshape(HD, 1))
    w2a = np.ascontiguousarray(np.concatenate([W2, b2[None, :]], axis=0))
    s64 = schemas.astype(np.float64)
    norms = np.maximum(np.sqrt((s64 * s64).sum(axis=1, keepdims=True)), EPS)
    schn = s64 / norms
    wsim = (w2a.astype(np.float64) @ schn.T).astype(np.float32)
    return w1s, b1c, w2a, wsim


def run_device(episodic_traces, W1, b1, W2, b2, schemas, **run_kwargs):
    """Compile + run the SPMD kernel; returns (partials [8,64,33], results)."""
    from concourse.bass_utils import run_bass_kernel_spmd

    nc = _get_nc()
    w1s, b1c, w2a, wsim = _prep_weights(W1, b1, W2, b2, schemas)
    in_maps = []
    for c in range(NCORES):
        xt = _transpose_shard(episodic_traces[c * RPC : (c + 1) * RPC])
        in_maps.append({"xt": xt, "w1s": w1s, "b1c": b1c, "w2a": w2a, "wsim": wsim})
    res = run_bass_kernel_spmd(nc, in_maps, list(range(NCORES)), **run_kwargs)
    partials = np.stack(
        [np.asarray(res.results[i]["partial"]) for i in range(NCORES)]
    )
    return partials, res


def kernel(episodic_traces, W1, b1, W2, b2, schemas, schema_usage):
    episodic_traces = np.asarray(episodic_traces, np.float32)
    W1 = np.asarray(W1, np.float32)
    b1 = np.asarray(b1, np.float32)
    W2 = np.asarray(W2, np.float32)
    b2 = np.asarray(b2, np.float32)
    schemas = np.asarray(schemas, np.float32)
    schema_usage = np.asarray(schema_usage, np.float32)

    partials, _ = run_device(episodic_traces, W1, b1, W2, b2, schemas)

    sums = partials[:, :, :SCHEMA_DIM].sum(axis=0)
    counts = partials[:, :, SCHEMA_DIM].sum(axis=0)

    nonempty = counts > 0
    target = sums / np.maximum(counts, np.float32(1.0))[:, None]
    delta = np.float32(LR) * (target - schemas) * nonempty[:, None].astype(np.float32)
    new_schemas = schemas + delta
    new_usage = schema_usage + counts
    norms = np.sqrt((delta * delta).sum(axis=-1))
    num_updated = np.int32(nonempty.sum())
    mean_update_norm = np.float32(norms.sum() / np.float32(max(1, int(num_updated))))
    return new_schemas, new_usage, num_updated, mean_update_norm
